# revision 11
# baseline (speedup 1.0000x reference)
"""Causal single-head attention on 8 TRN2 NeuronCores (v2).

Problem: x:(S=4096, B=4, E=5) f32; Wk/Wq/Wv:(5,64), bk/bq/bv:(64,).
  K/Q/V = x@W + b per batch; scores = K.Q^T/8 (keys i, queries j), causal
  (key i attends query j iff i <= j), softmax over keys per query, out =
  sum_i V[i]*P[i,j] -> (S, B, 64).

Key algebra: scores = X6 @ M @ X6^T with X6 = [x | 1] (S,6) and
M = Wk6 @ Wq6^T / 8 (6x6).  The host precomputes Y = M @ X6^T (6,S), so the
device mm1 contracts over just 6 dims: scores_block = X6k_block^T? no --
st[key, q] = sum_c X6[key, c] * Y[c, q]; lhsT = xt6k[:, kblk] (6,128),
rhs = y6q (6, 512).  No on-device K/Q projections at all.

Sharding: 8 cores = 4 batches x 2 query-stripe sets (parity 0 -> query
tiles {0,1024,2048,3072}, parity 1 -> {512,1536,2560,3584}).  One SPMD
graph, static per-slot key-block profile fcnt=(4,12,20,28); per-core
differences are input data only (slack key blocks have zeroed V-side x6,
so they contribute nothing to numerator or denominator).

Device computes only XP[c, q] = sum_k x6[k, c] * exp(s_kq) (12 rows per
slot: even-unit half + odd-unit half).  Host epilogue applies Wv6 and the
softmax normalization: out = (Wv6^T @ XP)[:64] / XP_ones_row.

PE usage: row-packed concurrent mm1 pairs (stationary at partitions 0:6
and 32:38), column-packed concurrent mm2 pairs (PSUM partitions 0:6 and
32:38), software-pipelined so mm1 of pair i+1 issues before mm2 of pair i
(PE never waits on the scalar-engine exp).
"""

import sys
from contextlib import ExitStack

import ml_dtypes
import numpy as np

for _p in ("/opt/trn_rl_repo", "/opt/pypackages"):
    if _p not in sys.path:
        sys.path.append(_p)

import concourse.bass as bass
import concourse.tile as tile
from concourse import bacc, mybir

F32 = mybir.dt.float32
BF16 = mybir.dt.bfloat16

S, B, E, NE = 4096, 4, 5, 64
N_CORES = 8
JT = 512          # query tile width
NSLOT = 4         # query tiles per core
FCNT = (4, 12, 20, 28)          # static full-unit (key-block) count per slot
F_OFF = (0, 4, 16, 36)          # cumulative offsets into x6v full blocks
NFULL = sum(FCNT)               # 64 blocks
NDIAG = NSLOT * 4               # 16 blocks
NBLK = NFULL + NDIAG            # 80 blocks in x6v
JOS_BY_PARITY = ((0, 1024, 2048, 3072), (512, 1536, 2560, 3584))
KQY_W = S + NSLOT * JT + NSLOT * JT   # xt6k | xt6q | y6q = 8192

_NC_CACHE = {}


def build_graph():
    nc = bacc.Bacc("TRN2", target_bir_lowering=False, debug=False)

    # kside = xt6k (6,4096); qside = [xt6q (6,2048) | y6q (6,2048)]
    kside = nc.declare_dram_parameter("kside", [6, S], BF16, isOutput=False)
    qside = nc.declare_dram_parameter("qside", [6, 2 * NSLOT * JT], BF16,
                                      isOutput=False)
    x6v = nc.declare_dram_parameter("x6v", [128, NBLK * 6], BF16, isOutput=False)
    xp48 = nc.declare_dram_parameter("xp48", [NSLOT * 12, JT], F32, isOutput=True)

    with tile.TileContext(nc) as tc, ExitStack() as ctx:
        consts = ctx.enter_context(tc.tile_pool(name="consts", bufs=1))
        psum = ctx.enter_context(tc.tile_pool(name="psum", bufs=2, space="PSUM"))
        sb = ctx.enter_context(tc.tile_pool(name="sb", bufs=2))

        # inputs, two partition-stripe copies (rows 0:6 / 32:38), ordered so
        # the first mm1's operands land first
        ks_sb = consts.tile([38, S], BF16)
        qs_sb = consts.tile([38, 2 * NSLOT * JT], BF16)
        x6v_sb = consts.tile([128, NBLK, 6], BF16)
        nc.sync.dma_start(out=ks_sb[0:6, :], in_=kside[:])
        nc.sync.dma_start(out=qs_sb[0:6, :], in_=qside[:])
        nc.sync.dma_start(out=ks_sb[32:38, :], in_=kside[:])
        nc.sync.dma_start(out=qs_sb[32:38, :], in_=qside[:])
        nc.sync.dma_start(
            out=x6v_sb[:], in_=x6v[:].rearrange("p (n c) -> p n c", c=6)
        )

        def kq(r0, c0, w):  # stripe r0 (0 or 32), cols [c0, c0+w)
            if c0 < S:
                return ks_sb[r0 : r0 + 6, c0 : c0 + w]
            c0 -= S
            return qs_sb[r0 : r0 + 6, c0 : c0 + w]

        XT6Q0, Y6Q0 = S, S + NSLOT * JT

        # diagonal causal masks, core-invariant: mask_d[p, q] = (p + 128d <= q)
        masks_f = consts.tile([128, 4 * JT], F32)
        nc.gpsimd.memset(masks_f[:], 1.0)
        for d in range(4):
            nc.gpsimd.affine_select(
                out=masks_f[:, d * JT : (d + 1) * JT],
                in_=masks_f[:, d * JT : (d + 1) * JT],
                compare_op=mybir.AluOpType.is_ge,
                fill=0.0,
                base=-128 * d,
                pattern=[[1, JT]],
                channel_multiplier=-1,
            )
        masks_sb = consts.tile([128, 4 * JT], BF16)
        nc.vector.tensor_copy(masks_sb[:], masks_f[:])

        # ---- build the global software-pipelined pair list ----
        # pair = (slot, lhsA_ap_args, lhsB_ap_args, xvA, xvB, mask_dp, start, stop)
        pairs = []
        for t in (3, 2, 1, 0):
            nf = FCNT[t]
            for up in range(nf // 2):
                ue, uo = 2 * up, 2 * up + 1
                pairs.append(
                    dict(
                        t=t,
                        lhsA=(0, ue * 128), lhsB=(32, uo * 128),
                        xvA=F_OFF[t] + ue, xvB=F_OFF[t] + uo,
                        mask_dp=None,
                        start=(up == 0), stop=False,
                    )
                )
            for dp in range(2):
                de, do = 2 * dp, 2 * dp + 1
                pairs.append(
                    dict(
                        t=t,
                        lhsA=(0, XT6Q0 + t * JT + de * 128),
                        lhsB=(32, XT6Q0 + t * JT + do * 128),
                        xvA=NFULL + 4 * t + de, xvB=NFULL + 4 * t + do,
                        mask_dp=dp,
                        start=False, stop=(dp == 1),
                    )
                )

        NP = len(pairs)  # 40
        st_tiles = [None] * NP
        pt_tiles = [None] * NP
        xp_by_slot = {}

        def emit_mm1(i):
            p = pairs[i]
            t = p["t"]
            st = psum.tile([128, 2 * JT], F32, tag="st", bufs=3)
            st_tiles[i] = st
            jcol = Y6Q0 + t * JT
            (rA, cA), (rB, cB) = p["lhsA"], p["lhsB"]
            nc.tensor.matmul(
                st[:, 0:JT], kq(rA, cA, 128), kq(0, jcol, JT),
                start=True, stop=True,
            )
            nc.tensor.matmul(
                st[:, JT : 2 * JT], kq(rB, cB, 128), kq(32, jcol, JT),
                start=True, stop=True,
            )
            pt = sb.tile([128, 2 * JT], BF16, tag="pt", bufs=3)
            pt_tiles[i] = pt
            nc.scalar.activation(
                pt[:], st[:], mybir.ActivationFunctionType.Exp
            )
            if p["mask_dp"] is not None:
                dp = p["mask_dp"]
                nc.vector.tensor_mul(
                    pt[:], pt[:], masks_sb[:, 2 * dp * JT : 2 * (dp + 1) * JT]
                )

        def emit_mm2(i):
            p = pairs[i]
            t = p["t"]
            if p["start"]:
                xp_by_slot[t] = psum.tile(
                    [38, JT], F32, tag="xp", bufs=2, name=f"xp{t}"
                )
            xp = xp_by_slot[t]
            pt = pt_tiles[i]
            nc.tensor.matmul(
                xp[0:6, :], x6v_sb[:, p["xvA"], :], pt[:, 0:JT],
                start=p["start"], stop=p["stop"], skip_group_check=True,
            )
            nc.tensor.matmul(
                xp[32:38, :], x6v_sb[:, p["xvB"], :], pt[:, JT : 2 * JT],
                start=p["start"], stop=p["stop"], skip_group_check=True,
            )
            if p["stop"]:
                xps = sb.tile([38, JT], F32, tag="xps", bufs=2)
                nc.vector.tensor_copy(xps[0:6, :], xp[0:6, :])
                if t == 0:  # final slot: scalar engine is free, copy in parallel
                    nc.scalar.copy(xps[32:38, :], xp[32:38, :])
                else:
                    nc.vector.tensor_copy(xps[32:38, :], xp[32:38, :])
                nc.sync.dma_start(
                    out=xp48[t * 12 : t * 12 + 6, :], in_=xps[0:6, :]
                )
                nc.sync.dma_start(
                    out=xp48[t * 12 + 6 : t * 12 + 12, :], in_=xps[32:38, :]
                )

        # software pipeline: mm1 runs one pair ahead of mm2
        emit_mm1(0)
        for i in range(1, NP):
            emit_mm1(i)
            emit_mm2(i - 1)
        emit_mm2(NP - 1)

    nc.compile()
    return nc


def make_in_maps(x, Wk, bk, Wq, bq, Wv, bv):
    """Build the 8 per-core input dicts from the full problem inputs."""
    x = np.asarray(x, np.float64)
    wk6 = np.vstack([np.asarray(Wk, np.float64), np.asarray(bk, np.float64)[None]])
    wq6 = np.vstack([np.asarray(Wq, np.float64), np.asarray(bq, np.float64)[None]])
    m66 = (wk6 @ wq6.T) / 8.0  # (6, 6): scores = X6 @ m66 @ X6^T

    in_maps = []
    for core in range(N_CORES):
        b, parity = core // 2, core % 2
        jos = JOS_BY_PARITY[parity]
        x6 = np.concatenate([x[:, b, :], np.ones((S, 1), np.float64)], axis=1)
        y6 = m66 @ x6.T  # (6, S)

        xt6q = np.concatenate([x6[jo : jo + JT].T for jo in jos], axis=1)
        y6q = np.concatenate([y6[:, jo : jo + JT] for jo in jos], axis=1)
        qside = np.concatenate([xt6q, y6q], axis=1)  # (6, 4096)

        x6v = np.zeros((128, NBLK, 6), np.float64)
        for t, jo in enumerate(jos):
            blk = x6[: FCNT[t] * 128].copy().reshape(FCNT[t], 128, 6)
            blk[jo // 128 :] = 0.0  # slack blocks: V-side zeroed
            x6v[:, F_OFF[t] : F_OFF[t] + FCNT[t], :] = blk.transpose(1, 0, 2)
            dblk = x6[jo : jo + JT].reshape(4, 128, 6)
            x6v[:, NFULL + 4 * t : NFULL + 4 * t + 4, :] = dblk.transpose(1, 0, 2)

        in_maps.append(
            {
                "kside": np.ascontiguousarray(x6.T).astype(ml_dtypes.bfloat16),
                "qside": np.ascontiguousarray(qside).astype(ml_dtypes.bfloat16),
                "x6v": np.ascontiguousarray(x6v.reshape(128, NBLK * 6)).astype(
                    ml_dtypes.bfloat16
                ),
            }
        )
    return in_maps


def assemble_output(results, Wv, bv):
    """Host epilogue: apply Wv6, normalize, stitch into (S, B, NE)."""
    wv6 = np.vstack([np.asarray(Wv, np.float64), np.asarray(bv, np.float64)[None]])
    out = np.zeros((S, B, NE), np.float32)
    for core in range(N_CORES):
        b, parity = core // 2, core % 2
        jos = JOS_BY_PARITY[parity]
        xp48 = np.asarray(results[core]["xp48"], np.float64)  # (48, 512)
        for t, jo in enumerate(jos):
            xp = xp48[t * 12 : t * 12 + 6] + xp48[t * 12 + 6 : t * 12 + 12]
            num = wv6.T @ xp  # (64, 512): Wv^T x-moments + bv * ones-row
            out[jo : jo + JT, b, :] = (num / xp[5]).T
    return out


def run_on_device(in_maps, trace=False):
    from concourse.bass_utils import run_bass_kernel_spmd

    if "nc" not in _NC_CACHE:
        _NC_CACHE["nc"] = build_graph()
    nc = _NC_CACHE["nc"]
    return run_bass_kernel_spmd(
        nc, in_maps, core_ids=list(range(N_CORES)), trace=trace
    )


def kernel(x, Wk, bk, Wq, bq, Wv, bv):
    in_maps = make_in_maps(x, Wk, bk, Wq, bq, Wv, bv)
    res = run_on_device(in_maps, trace=False)
    return assemble_output(res.results, Wv, bv)


# revision 14
# speedup vs baseline: 1.0156x; 1.0156x over previous
"""Causal single-head attention on 8 TRN2 NeuronCores (v2).

Problem: x:(S=4096, B=4, E=5) f32; Wk/Wq/Wv:(5,64), bk/bq/bv:(64,).
  K/Q/V = x@W + b per batch; scores = K.Q^T/8 (keys i, queries j), causal
  (key i attends query j iff i <= j), softmax over keys per query, out =
  sum_i V[i]*P[i,j] -> (S, B, 64).

Key algebra: scores = X6 @ M @ X6^T with X6 = [x | 1] (S,6) and
M = Wk6 @ Wq6^T / 8 (6x6).  The host precomputes Y = M @ X6^T (6,S), so the
device mm1 contracts over just 6 dims: scores_block = X6k_block^T? no --
st[key, q] = sum_c X6[key, c] * Y[c, q]; lhsT = xt6k[:, kblk] (6,128),
rhs = y6q (6, 512).  No on-device K/Q projections at all.

Sharding: 8 cores = 4 batches x 2 query-stripe sets (parity 0 -> query
tiles {0,1024,2048,3072}, parity 1 -> {512,1536,2560,3584}).  One SPMD
graph, static per-slot key-block profile fcnt=(4,12,20,28); per-core
differences are input data only (slack key blocks have zeroed V-side x6,
so they contribute nothing to numerator or denominator).

Device computes only XP[c, q] = sum_k x6[k, c] * exp(s_kq) (12 rows per
slot: even-unit half + odd-unit half).  Host epilogue applies Wv6 and the
softmax normalization: out = (Wv6^T @ XP)[:64] / XP_ones_row.

PE usage: row-packed concurrent mm1 pairs (stationary at partitions 0:6
and 32:38), column-packed concurrent mm2 pairs (PSUM partitions 0:6 and
32:38), software-pipelined so mm1 of pair i+1 issues before mm2 of pair i
(PE never waits on the scalar-engine exp).
"""

import sys
from contextlib import ExitStack

import ml_dtypes
import numpy as np

for _p in ("/opt/trn_rl_repo", "/opt/pypackages"):
    if _p not in sys.path:
        sys.path.append(_p)

import concourse.bass as bass
import concourse.tile as tile
from concourse import bacc, mybir

F32 = mybir.dt.float32
BF16 = mybir.dt.bfloat16

S, B, E, NE = 4096, 4, 5, 64
N_CORES = 8
JT = 512          # query tile width
NSLOT = 4         # query tiles per core
FCNT = (4, 12, 20, 28)          # static full-unit (key-block) count per slot
F_OFF = (0, 4, 16, 36)          # cumulative offsets into x6v full blocks
NFULL = sum(FCNT)               # 64 blocks
NDIAG = NSLOT * 4               # 16 blocks
NBLK = NFULL + NDIAG            # 80 blocks in x6v
JOS_BY_PARITY = ((0, 1024, 2048, 3072), (512, 1536, 2560, 3584))
KQY_W = S + NSLOT * JT + NSLOT * JT   # xt6k | xt6q | y6q = 8192

_NC_CACHE = {}


def build_graph():
    nc = bacc.Bacc("TRN2", target_bir_lowering=False, debug=False)

    # kside = xt6k (6,4096); y6q/xt6q (6,2048) each
    kside = nc.declare_dram_parameter("kside", [6, S], BF16, isOutput=False)
    y6q = nc.declare_dram_parameter("y6q", [6, NSLOT * JT], BF16, isOutput=False)
    xt6q = nc.declare_dram_parameter("xt6q", [6, NSLOT * JT], BF16, isOutput=False)
    x6v = nc.declare_dram_parameter("x6v", [128, NBLK * 6], BF16, isOutput=False)
    xp48 = nc.declare_dram_parameter("xp48", [NSLOT * 12, JT], F32, isOutput=True)

    with tile.TileContext(nc) as tc, ExitStack() as ctx:
        consts = ctx.enter_context(tc.tile_pool(name="consts", bufs=1))
        psum = ctx.enter_context(tc.tile_pool(name="psum", bufs=2, space="PSUM"))
        sb = ctx.enter_context(tc.tile_pool(name="sb", bufs=2))

        # inputs, two partition-stripe copies (rows 0:6 / 32:38).  Issue on
        # both HWDGE rings (sync + scalar) ordered by first use: the first
        # mm1 pair needs only kside/y6q stripe 0.
        ks_sb = consts.tile([38, S], BF16)
        yq_sb = consts.tile([38, NSLOT * JT], BF16)
        xq_sb = consts.tile([38, NSLOT * JT], BF16)
        x6v_sb = consts.tile([128, NBLK, 6], BF16)
        nc.sync.dma_start(out=ks_sb[0:6, :], in_=kside[:])
        nc.scalar.dma_start(out=yq_sb[0:6, :], in_=y6q[:])
        nc.sync.dma_start(out=ks_sb[32:38, :], in_=kside[:])
        nc.scalar.dma_start(out=yq_sb[32:38, :], in_=y6q[:])
        nc.sync.dma_start(
            out=x6v_sb[:], in_=x6v[:].rearrange("p (n c) -> p n c", c=6)
        )
        nc.scalar.dma_start(out=xq_sb[0:6, :], in_=xt6q[:])
        nc.sync.dma_start(out=xq_sb[32:38, :], in_=xt6q[:])

        def kq(r0, c0, w):  # stripe r0 (0 or 32), cols [c0, c0+w)
            if c0 < S:
                return ks_sb[r0 : r0 + 6, c0 : c0 + w]
            if c0 < S + NSLOT * JT:
                c0 -= S
                return xq_sb[r0 : r0 + 6, c0 : c0 + w]

            c0 -= S + NSLOT * JT
            return yq_sb[r0 : r0 + 6, c0 : c0 + w]

        XT6Q0, Y6Q0 = S, S + NSLOT * JT

        # diagonal causal masks, core-invariant: mask_d[p, q] = (p + 128d <= q)
        masks_f = consts.tile([128, 4 * JT], F32)
        nc.gpsimd.memset(masks_f[:], 1.0)
        for d in range(4):
            nc.gpsimd.affine_select(
                out=masks_f[:, d * JT : (d + 1) * JT],
                in_=masks_f[:, d * JT : (d + 1) * JT],
                compare_op=mybir.AluOpType.is_ge,
                fill=0.0,
                base=-128 * d,
                pattern=[[1, JT]],
                channel_multiplier=-1,
            )
        masks_sb = consts.tile([128, 4 * JT], BF16)
        nc.vector.tensor_copy(masks_sb[:], masks_f[:])

        # ---- build the global software-pipelined pair list ----
        # pair = (slot, lhsA_ap_args, lhsB_ap_args, xvA, xvB, mask_dp, start, stop)
        pairs = []
        for t in (3, 2, 1, 0):
            nf = FCNT[t]
            for up in range(nf // 2):
                ue, uo = 2 * up, 2 * up + 1
                pairs.append(
                    dict(
                        t=t,
                        lhsA=(0, ue * 128), lhsB=(32, uo * 128),
                        xvA=F_OFF[t] + ue, xvB=F_OFF[t] + uo,
                        mask_dp=None,
                        start=(up == 0), stop=False,
                    )
                )
            for dp in range(2):
                de, do = 2 * dp, 2 * dp + 1
                pairs.append(
                    dict(
                        t=t,
                        lhsA=(0, XT6Q0 + t * JT + de * 128),
                        lhsB=(32, XT6Q0 + t * JT + do * 128),
                        xvA=NFULL + 4 * t + de, xvB=NFULL + 4 * t + do,
                        mask_dp=dp,
                        start=False, stop=(dp == 1),
                    )
                )

        NP = len(pairs)  # 40
        st_tiles = [None] * NP
        pt_tiles = [None] * NP
        xp_by_slot = {}

        def emit_mm1(i):
            p = pairs[i]
            t = p["t"]
            st = psum.tile([128, 2 * JT], F32, tag="st", bufs=3)
            st_tiles[i] = st
            jcol = Y6Q0 + t * JT
            (rA, cA), (rB, cB) = p["lhsA"], p["lhsB"]
            nc.tensor.matmul(
                st[:, 0:JT], kq(rA, cA, 128), kq(0, jcol, JT),
                start=True, stop=True,
            )
            nc.tensor.matmul(
                st[:, JT : 2 * JT], kq(rB, cB, 128), kq(32, jcol, JT),
                start=True, stop=True,
            )
            pt = sb.tile([128, 2 * JT], BF16, tag="pt", bufs=3)
            pt_tiles[i] = pt
            nc.scalar.activation(
                pt[:], st[:], mybir.ActivationFunctionType.Exp
            )
            if p["mask_dp"] is not None:
                dp = p["mask_dp"]
                nc.vector.tensor_mul(
                    pt[:], pt[:], masks_sb[:, 2 * dp * JT : 2 * (dp + 1) * JT]
                )

        def emit_mm2(i):
            p = pairs[i]
            t = p["t"]
            if p["start"]:
                xp_by_slot[t] = psum.tile(
                    [38, JT], F32, tag="xp", bufs=2, name=f"xp{t}"
                )
            xp = xp_by_slot[t]
            pt = pt_tiles[i]
            nc.tensor.matmul(
                xp[0:6, :], x6v_sb[:, p["xvA"], :], pt[:, 0:JT],
                start=p["start"], stop=p["stop"], skip_group_check=True,
            )
            nc.tensor.matmul(
                xp[32:38, :], x6v_sb[:, p["xvB"], :], pt[:, JT : 2 * JT],
                start=p["start"], stop=p["stop"], skip_group_check=True,
            )
            if p["stop"]:
                xps = sb.tile([38, JT], F32, tag="xps", bufs=2)
                nc.vector.tensor_copy(xps[0:6, :], xp[0:6, :])
                if t == 0:  # final slot: scalar engine is free, copy in parallel
                    nc.scalar.copy(xps[32:38, :], xp[32:38, :])
                else:
                    nc.vector.tensor_copy(xps[32:38, :], xp[32:38, :])
                nc.sync.dma_start(
                    out=xp48[t * 12 : t * 12 + 6, :], in_=xps[0:6, :]
                )
                nc.sync.dma_start(
                    out=xp48[t * 12 + 6 : t * 12 + 12, :], in_=xps[32:38, :]
                )

        # software pipeline: mm1 runs one pair ahead of mm2
        emit_mm1(0)
        for i in range(1, NP):
            emit_mm1(i)
            emit_mm2(i - 1)
        emit_mm2(NP - 1)

    nc.compile()
    return nc


def make_in_maps(x, Wk, bk, Wq, bq, Wv, bv):
    """Build the 8 per-core input dicts from the full problem inputs."""
    x = np.asarray(x, np.float64)
    wk6 = np.vstack([np.asarray(Wk, np.float64), np.asarray(bk, np.float64)[None]])
    wq6 = np.vstack([np.asarray(Wq, np.float64), np.asarray(bq, np.float64)[None]])
    m66 = (wk6 @ wq6.T) / 8.0  # (6, 6): scores = X6 @ m66 @ X6^T

    in_maps = []
    for core in range(N_CORES):
        b, parity = core // 2, core % 2
        jos = JOS_BY_PARITY[parity]
        x6 = np.concatenate([x[:, b, :], np.ones((S, 1), np.float64)], axis=1)
        y6 = m66 @ x6.T  # (6, S)

        xt6q = np.concatenate([x6[jo : jo + JT].T for jo in jos], axis=1)
        y6q = np.concatenate([y6[:, jo : jo + JT] for jo in jos], axis=1)

        x6v = np.zeros((128, NBLK, 6), np.float64)
        for t, jo in enumerate(jos):
            blk = x6[: FCNT[t] * 128].copy().reshape(FCNT[t], 128, 6)
            blk[jo // 128 :] = 0.0  # slack blocks: V-side zeroed
            x6v[:, F_OFF[t] : F_OFF[t] + FCNT[t], :] = blk.transpose(1, 0, 2)
            dblk = x6[jo : jo + JT].reshape(4, 128, 6)
            x6v[:, NFULL + 4 * t : NFULL + 4 * t + 4, :] = dblk.transpose(1, 0, 2)

        in_maps.append(
            {
                "kside": np.ascontiguousarray(x6.T).astype(ml_dtypes.bfloat16),
                "y6q": np.ascontiguousarray(y6q).astype(ml_dtypes.bfloat16),
                "xt6q": np.ascontiguousarray(xt6q).astype(ml_dtypes.bfloat16),
                "x6v": np.ascontiguousarray(x6v.reshape(128, NBLK * 6)).astype(
                    ml_dtypes.bfloat16
                ),
            }
        )
    return in_maps


def assemble_output(results, Wv, bv):
    """Host epilogue: apply Wv6, normalize, stitch into (S, B, NE)."""
    wv6 = np.vstack([np.asarray(Wv, np.float64), np.asarray(bv, np.float64)[None]])
    out = np.zeros((S, B, NE), np.float32)
    for core in range(N_CORES):
        b, parity = core // 2, core % 2
        jos = JOS_BY_PARITY[parity]
        xp48 = np.asarray(results[core]["xp48"], np.float64)  # (48, 512)
        for t, jo in enumerate(jos):
            xp = xp48[t * 12 : t * 12 + 6] + xp48[t * 12 + 6 : t * 12 + 12]
            num = wv6.T @ xp  # (64, 512): Wv^T x-moments + bv * ones-row
            out[jo : jo + JT, b, :] = (num / xp[5]).T
    return out


def run_on_device(in_maps, trace=False):
    from concourse.bass_utils import run_bass_kernel_spmd

    if "nc" not in _NC_CACHE:
        _NC_CACHE["nc"] = build_graph()
    nc = _NC_CACHE["nc"]
    return run_bass_kernel_spmd(
        nc, in_maps, core_ids=list(range(N_CORES)), trace=trace
    )


def kernel(x, Wk, bk, Wq, bq, Wv, bv):
    in_maps = make_in_maps(x, Wk, bk, Wq, bq, Wv, bv)
    res = run_on_device(in_maps, trace=False)
    return assemble_output(res.results, Wv, bv)
